# revision 1
# baseline (speedup 1.0000x reference)
"""LDA head forward on 8 Trainium2 NeuronCores (Bass/Tile).

Data-parallel over B=4096 rows. Per core (B_l=512):
  phase A: one-hot via iota+is_equal, partial stats via PE matmuls:
           S1T = Z_l^T @ onehot  [D,C], counts = 1^T @ onehot [1,C],
           ZtZ = Z_l^T @ Z_l     [D,D]
  one fused AllReduce of (S1T | counts | ZtZ)  (~330KB), preceded by a
  tiny warm-up AllReduce that absorbs the cross-core entry barrier while
  phase A compute runs.
  phase B (replicated): pooled covariance via the algebraic identity
           pooled = (ZtZ - sum_c w_c S1_c S1_c^T) / total + eps*I,
           w_c = (n_c + 2eps)/counts_c^2;
           precision P = pooled^-1 via Newton-Schulz (X1 = 2cI - c^2 A
           computed directly on DVE, then 3 matmul iterations).
  phase C: scores[b,c] = logprior_c - 0.5 q_b - 0.5 r_c + (Z P mean^T)[b,c]
           (no [B,C,D] tensor is ever materialized)

float32r (tf32) is used only where the matmul free dim is 512 (full-rate
PE); all N=128 matmuls stay exact fp32. Per-class scalar math runs in
[128, 4] partition layout (single-lane [1, 512] row ops are ~7x slower);
the two row vectors needed for broadcasts are produced by PE transposes.
"""

import numpy as np

import concourse.bacc as bacc
import concourse.bass as bass
import concourse.mybir as mybir
import concourse.tile as tile
from concourse.bass_utils import run_bass_kernel_spmd

f32 = mybir.dt.float32
f32r = mybir.dt.float32r
AL = mybir.AluOpType
AF = mybir.ActivationFunctionType

M = 8            # cores
B = 4096
D = 128
C = 512
BL = B // M      # 512 rows per core
KC = BL // 128   # 4 chunks of 128 rows
EPS = 1e-5
TOTAL = float(B) + C * EPS
LN_TOTAL = float(np.log(np.float32(TOTAL)))
NS_C = 1.05      # Newton-Schulz init scale; X1 = 2c*I - c^2*A
NS_ITERS = 3     # matmul iterations after the direct X1
WARM_CC = True   # tiny leading AllReduce to absorb the entry barrier


def build_program():
    nc = bacc.Bacc("TRN2", target_bir_lowering=False, debug=False, num_devices=M)
    z_d = nc.dram_tensor("z", [BL, D], f32, kind="ExternalInput").ap()
    zt_d = nc.dram_tensor("zT", [D, BL], f32, kind="ExternalInput").ap()
    y_d = nc.dram_tensor("y", [128, KC], f32, kind="ExternalInput").ap()
    out_d = nc.dram_tensor("scores", [BL, C], f32, kind="ExternalOutput").ap()

    with tile.TileContext(nc) as tc:
        _body(tc, out_d, z_d, zt_d, y_d)
    nc.compile()
    return nc


def _body(tc, out_d, z_d, zt_d, y_d):
    nc = tc.nc
    rg = [list(range(M))]
    with (
        tc.tile_pool(name="const", bufs=1) as const,
        tc.tile_pool(name="io", bufs=1) as io,
        tc.tile_pool(name="sb", bufs=1) as sb,
        tc.tile_pool(name="rows", bufs=1) as rows,
        tc.tile_pool(name="dram", bufs=1, space="DRAM") as dram,
    ):
        # ---- warm-up collective: sync the 8 cores while phase A runs ----
        if WARM_CC:
            wsrc = rows.tile([1, 64], f32)
            nc.gpsimd.memset(wsrc[:], 0.0)
            warm_in = dram.tile([1, 64], f32)
            warm_out = dram.tile([1, 64], f32, addr_space="Shared")
            nc.sync.dma_start(warm_in[:], wsrc[:])
            nc.gpsimd.collective_compute(
                "AllReduce", AL.add, replica_groups=rg,
                ins=[warm_in.opt()], outs=[warm_out.opt()],
            )

        # ---- constants ----
        ident = const.tile([128, 128], f32)
        nc.gpsimd.memset(ident[:], 0.0)
        nc.gpsimd.affine_select(
            out=ident[:], in_=ident[:], compare_op=AL.not_equal, fill=1.0,
            base=0, pattern=[[-1, 128]], channel_multiplier=1,
        )
        eps_eye = const.tile([128, 128], f32)
        nc.gpsimd.memset(eps_eye[:], 0.0)
        nc.gpsimd.affine_select(
            out=eps_eye[:], in_=eps_eye[:], compare_op=AL.not_equal, fill=EPS,
            base=0, pattern=[[-1, 128]], channel_multiplier=1,
        )
        # 2c*I for the direct first Newton-Schulz step
        tc_eye = const.tile([128, 128], f32)
        nc.gpsimd.memset(tc_eye[:], 0.0)
        nc.gpsimd.affine_select(
            out=tc_eye[:], in_=tc_eye[:], compare_op=AL.not_equal, fill=2.0 * NS_C,
            base=0, pattern=[[-1, 128]], channel_multiplier=1,
        )
        iota = const.tile([128, C], f32)
        nc.gpsimd.iota(
            iota[:], pattern=[[1, C]], base=0, channel_multiplier=0,
            allow_small_or_imprecise_dtypes=True,
        )
        ones_f = const.tile([128, 1], f32)
        nc.gpsimd.memset(ones_f[:], 1.0)
        ones_r = const.tile([128, 1], f32r)
        nc.vector.tensor_copy(ones_r[:], ones_f[:])
        # preload ACT tables (Ln, Sqrt) off the critical path
        tbl = rows.tile([1, 1], f32)
        nc.scalar.activation(tbl[:], ones_f[0:1, :], AF.Ln)
        tbl2 = rows.tile([1, 1], f32)
        nc.scalar.activation(tbl2[:], ones_f[0:1, :], AF.Sqrt)

        # ---- inputs ----
        z = io.tile([128, KC, 128], f32)      # z rows b=k*128+p at [p, k, :]
        nc.sync.dma_start(z[:], z_d.rearrange("(k p) d -> p k d", p=128))
        zt = io.tile([D, BL], f32)            # z^T
        nc.sync.dma_start(zt[:], zt_d)
        yv = io.tile([128, KC], f32)
        nc.sync.dma_start(yv[:], y_d)
        zr = io.tile([128, KC, 128], f32r)    # tf32-rounded copies
        nc.vector.tensor_copy(zr[:], z[:])
        ztr = io.tile([D, BL], f32r)
        nc.vector.tensor_copy(ztr[:], zt[:])

        # ---- phase A: local stats ----
        ar_in = dram.tile([161, C], f32)
        ar_out = dram.tile([161, C], f32, addr_space="Shared")
        with tc.tile_pool(name="psA", bufs=1, space="PSUM") as psA:
            ps_s1t = psA.tile([128, C], f32)
            ps_cnt = psA.tile([1, C], f32)
            ps_ztz = psA.tile([128, 128], f32)
            for k in range(KC):
                oh = sb.tile([128, C], f32r, tag="oh", bufs=2)
                nc.vector.tensor_scalar(
                    out=oh[:], in0=iota[:], scalar1=yv[:, k : k + 1], scalar2=None,
                    op0=AL.is_equal,
                )
                st = k == 0
                sp = k == KC - 1
                nc.tensor.matmul(ps_s1t[:], lhsT=zr[:, k, :], rhs=oh[:], start=st, stop=sp)
                nc.tensor.matmul(ps_cnt[:], lhsT=ones_r[:], rhs=oh[:], start=st, stop=sp)
                nc.tensor.matmul(ps_ztz[:], lhsT=z[:, k, :], rhs=z[:, k, :], start=st, stop=sp)

            # pack [161, 512]: rows 0:128 S1T, row 128 counts, rows 129:161 ZtZ
            s1t_l = sb.tile([128, C], f32)
            nc.vector.tensor_copy(s1t_l[:], ps_s1t[:])
            cnt_l = rows.tile([1, C], f32)
            nc.vector.tensor_copy(cnt_l[:], ps_cnt[:])
            ztz_l = sb.tile([128, 128], f32)
            nc.vector.tensor_copy(ztz_l[:], ps_ztz[:])
            nc.sync.dma_start(ar_in[0:128, :], s1t_l[:])
            nc.sync.dma_start(ar_in[128:129, :], cnt_l[:])
            nc.sync.dma_start(
                ar_in[129:161, :].rearrange("a (b c) -> (a b) c", b=4), ztz_l[:]
            )

        nc.gpsimd.collective_compute(
            "AllReduce", AL.add, replica_groups=rg,
            ins=[ar_in.opt()], outs=[ar_out.opt()],
        )

        # ---- phase B ----
        s1t = sb.tile([128, C], f32)
        nc.sync.dma_start(s1t[:], ar_out[0:128, :])
        ztz = sb.tile([128, 128], f32)
        nc.sync.dma_start(
            ztz[:], ar_out[129:161, :].rearrange("a (b c) -> (a b) c", b=4)
        )
        # counts in [128, 4] partition layout: cp[p, j] = counts[j*128 + p]
        cp_raw = io.tile([128, KC], f32)
        nc.sync.dma_start(
            cp_raw[:], ar_out[128:129, :].rearrange("a (k p) -> (a p) k", p=128)
        )

        # per-class scalar math, [128, 4] layout (fast: 128 lanes)
        cnts = io.tile([128, KC], f32)
        nc.vector.tensor_scalar(
            out=cnts[:], in0=cp_raw[:], scalar1=EPS, scalar2=None, op0=AL.add
        )
        rcp = io.tile([128, KC], f32)
        nc.vector.reciprocal(rcp[:], cnts[:])
        lncp = io.tile([128, KC], f32)
        nc.scalar.activation(lncp[:], cnts[:], AF.Ln)
        cpe = io.tile([128, KC], f32)
        nc.vector.tensor_scalar(
            out=cpe[:], in0=cnts[:], scalar1=EPS, scalar2=None, op0=AL.add
        )
        swp = io.tile([128, KC], f32)
        nc.scalar.activation(swp[:], cpe[:], AF.Sqrt)
        nc.vector.tensor_tensor(swp[:], swp[:], rcp[:], op=AL.mult)

        with tc.tile_pool(name="psB1", bufs=1, space="PSUM") as psB1:
            # rc and ln(counts) as [1, 512] rows via PE transposes
            ps_rcrow = psB1.tile([1, C], f32)
            ps_lnrow = psB1.tile([1, C], f32)
            for j in range(KC):
                nc.tensor.transpose(
                    ps_rcrow[0:1, j * 128 : (j + 1) * 128], rcp[:, j : j + 1], ident[:]
                )
                nc.tensor.transpose(
                    ps_lnrow[0:1, j * 128 : (j + 1) * 128], lncp[:, j : j + 1], ident[:]
                )
            rc_row = rows.tile([1, C], f32)
            nc.vector.tensor_copy(rc_row[:], ps_rcrow[:])
            ln_row = rows.tile([1, C], f32)
            nc.vector.tensor_copy(ln_row[:], ps_lnrow[:])

            # W2 = sum_c w_c S1_c S1_c^T via Gram of V = diag(sqrtw) S1
            ps_w2 = psB1.tile([128, 128], f32)
            for j in range(KC):
                ps_tr = psB1.tile([128, 128], f32, tag="tr", bufs=2)
                nc.tensor.transpose(ps_tr[:], s1t[:, j * 128 : (j + 1) * 128], ident[:])
                vj = sb.tile([128, 128], f32, tag="vj", bufs=2)
                nc.vector.tensor_scalar(
                    out=vj[:], in0=ps_tr[:], scalar1=swp[:, j : j + 1], scalar2=None,
                    op0=AL.mult,
                )
                nc.tensor.matmul(
                    ps_w2[:], lhsT=vj[:], rhs=vj[:], start=(j == 0), stop=(j == KC - 1)
                )

            pooled = sb.tile([128, 128], f32)
            nc.vector.tensor_tensor(pooled[:], ztz[:], ps_w2[:], op=AL.subtract)
            nc.vector.scalar_tensor_tensor(
                out=pooled[:], in0=pooled[:], scalar=1.0 / TOTAL, in1=eps_eye[:],
                op0=AL.mult, op1=AL.add,
            )

        rc_b = sb.tile([128, C], f32)
        nc.gpsimd.partition_broadcast(rc_b[:], rc_row[:])
        meanT = sb.tile([128, C], f32)
        nc.vector.tensor_tensor(meanT[:], s1t[:], rc_b[:], op=AL.mult)

        with tc.tile_pool(name="psB2", bufs=1, space="PSUM") as psB2:
            # Newton-Schulz: X1 = 2c I - c^2 A directly, then X <- X(2I - A X)
            x_cur = sb.tile([128, 128], f32, tag="X", bufs=2)
            nc.vector.scalar_tensor_tensor(
                out=x_cur[:], in0=pooled[:], scalar=-NS_C * NS_C, in1=tc_eye[:],
                op0=AL.mult, op1=AL.add,
            )
            for i in range(NS_ITERS):
                ps_t = psB2.tile([128, 128], f32, tag="T", bufs=1)
                nc.tensor.matmul(ps_t[:], lhsT=pooled[:], rhs=x_cur[:], start=True, stop=True)
                t_s = sb.tile([128, 128], f32, tag="Ts", bufs=2)
                nc.vector.tensor_copy(t_s[:], ps_t[:])
                ps_u = psB2.tile([128, 128], f32, tag="U", bufs=1)
                nc.tensor.matmul(ps_u[:], lhsT=x_cur[:], rhs=t_s[:], start=True, stop=True)
                x_new = sb.tile([128, 128], f32, tag="X", bufs=2)
                nc.vector.scalar_tensor_tensor(
                    out=x_new[:], in0=x_cur[:], scalar=2.0, in1=ps_u[:],
                    op0=AL.mult, op1=AL.subtract,
                )
                x_cur = x_new

            # Pmt = P @ meanT (fp32, N=512), rounded copy for phase C
            ps_pmt = psB2.tile([128, C], f32)
            nc.tensor.matmul(ps_pmt[:], lhsT=x_cur[:], rhs=meanT[:], start=True, stop=True)
            pmt_r = sb.tile([128, C], f32r)
            nc.vector.tensor_copy(pmt_r[:], ps_pmt[:])

            # 0.5*r row: colsum(0.5 * meanT * Pmt) via ones-matmul
            prod = sb.tile([128, C], f32r)
            nc.vector.scalar_tensor_tensor(
                out=prod[:], in0=meanT[:], scalar=0.5, in1=ps_pmt[:],
                op0=AL.mult, op1=AL.mult,
            )
            ps_r = psB2.tile([1, C], f32)
            nc.tensor.matmul(ps_r[:], lhsT=ones_r[:], rhs=prod[:], start=True, stop=True)

            # rowcombo = ln(counts) - ln(total) - 0.5 r
            rowcombo = rows.tile([1, C], f32)
            nc.vector.scalar_tensor_tensor(
                out=rowcombo[:], in0=ln_row[:], scalar=-LN_TOTAL, in1=ps_r[:],
                op0=AL.add, op1=AL.subtract,
            )
        rc_bc = sb.tile([128, C], f32)
        nc.gpsimd.partition_broadcast(rc_bc[:], rowcombo[:])

        # ---- phase C: scores ----
        with tc.tile_pool(name="psC", bufs=1, space="PSUM") as psC:
            for k in range(KC):
                ps_g = psC.tile([128, C], f32, tag="g", bufs=2)
                nc.tensor.matmul(
                    ps_g[:], lhsT=ztr[:, k * 128 : (k + 1) * 128], rhs=pmt_r[:],
                    start=True, stop=True,
                )
                ps_zp = psC.tile([128, 128], f32, tag="zp", bufs=2)
                nc.tensor.matmul(
                    ps_zp[:], lhsT=zt[:, k * 128 : (k + 1) * 128], rhs=x_cur[:],
                    start=True, stop=True,
                )
                zpz = sb.tile([128, 128], f32, tag="zpz", bufs=2)
                qm = sb.tile([128, 1], f32, tag="qm", bufs=2)
                nc.vector.scalar_tensor_tensor(
                    out=zpz[:], in0=ps_zp[:], scalar=-0.5, in1=z[:, k, :],
                    op0=AL.mult, op1=AL.mult, accum_out=qm[:],
                )
                oc = sb.tile([128, C], f32, tag="oc", bufs=2)
                nc.vector.scalar_tensor_tensor(
                    out=oc[:], in0=ps_g[:], scalar=qm[:], in1=rc_bc[:],
                    op0=AL.add, op1=AL.add,
                )
                nc.sync.dma_start(out_d[k * 128 : (k + 1) * 128, :], oc[:])


_NC_CACHE = {}


def _get_nc():
    if "nc" not in _NC_CACHE:
        _NC_CACHE["nc"] = build_program()
    return _NC_CACHE["nc"]


def kernel(z, y):
    z = np.ascontiguousarray(np.asarray(z), dtype=np.float32)
    y = np.asarray(y).astype(np.float32)
    assert z.shape == (B, D) and y.shape == (B,)
    nc = _get_nc()
    in_maps = []
    for m in range(M):
        zs = z[m * BL : (m + 1) * BL]
        ys = y[m * BL : (m + 1) * BL]
        in_maps.append(
            {
                "z": zs,
                "zT": np.ascontiguousarray(zs.T),
                "y": np.ascontiguousarray(ys.reshape(KC, 128).T),
            }
        )
    res = run_bass_kernel_spmd(nc, in_maps, list(range(M)), trace=False)
    out = np.concatenate([res.results[m]["scores"] for m in range(M)], axis=0)
    return out.astype(np.float32)


if __name__ == "__main__":
    rng = np.random.default_rng(0)
    z = rng.standard_normal((B, D), dtype=np.float32)
    y = rng.integers(0, C, size=B).astype(np.int32)
    out = kernel(z, y)
    print("scores:", out.shape, out.dtype, out[:2, :4])



# revision 6
# speedup vs baseline: 1.6911x; 1.6911x over previous
"""LDA head forward on 8 Trainium2 NeuronCores (Bass/Tile).

Fully replicated statistics — ZERO collectives. Rationale: the SPMD launch
skews core start times by tens of us, so any cross-core barrier (AllReduce)
makes every core wait for the last-launched one. Instead each core
redundantly computes the full-batch statistics (phase A over all B=4096
rows) and only the [B_l, C] scoring (phase C) is sharded. No core ever
waits on another.

Per core:
  phase A: for each of 32 row-chunks: one-hot via iota+is_equal (fp16),
           PE accumulates S1T = Z^T OH [D,C] and ZtZ = Z^T Z [D,D] in PSUM.
           counts come from the DVE free-dim reduction (accum_out) over
           class-partition one-hot planes [c_part, b] — no PE stream needed.
  phase B: pooled = (ZtZ - sum_c w_c S1_c S1_c^T)/total + eps*I with
           w_c = (counts_c + eps)/counts_c^2 (asymmetric Gram: no sqrt),
           precision P via Newton-Schulz in fp16 (X1 = 2c I - c^2 A direct),
           Pmt = P @ (S1T diag(1/counts)).
  phase C: scores[b,c] = lnprior_c - 0.5 r_c - 0.5 q_b + (Z P mean^T)[b,c]
           on the core's own 512 rows.

fp16 is used for all PE streams (1 cycle/row vs 4 for fp32 and for tf32 on
<256-col outputs). All accumulation stays fp32 in PSUM. Per-class scalars
that can overflow fp16 (1/counts = 1e5 for an empty class) are clamped; the
clamped values only ever multiply exact zeros (S1_c = 0 for empty c).
Row/col broadcasts are PE outer products with one-hot selector stationaries
(no gpsimd partition_broadcast on the critical path).
"""

import numpy as np

import concourse.bacc as bacc
import concourse.mybir as mybir
import concourse.tile as tile
from concourse.bass_utils import run_bass_kernel_spmd

f32 = mybir.dt.float32
f16 = mybir.dt.float16
AL = mybir.AluOpType
AF = mybir.ActivationFunctionType

M = 8            # cores
B = 4096
D = 128
C = 512
BL = B // M      # 512 rows per core
KC = BL // 128   # 4 own chunks of 128 rows
KA = B // 128    # 32 total chunks
EPS = 1e-5
TOTAL = float(B) + C * EPS
LN_TOTAL = float(np.log(np.float64(TOTAL)))
NS_C = 1.05      # Newton-Schulz init scale; X1 = 2c*I - c^2*A
NS_ITERS = 3
CLAMP = 60000.0  # fp16-safe cap for per-class reciprocals/weights
NZH = 8          # z arrives in NZH separate DMA tiles so PE can start early
KPT = KA // NZH  # chunks per z tile


def build_program():
    nc = bacc.Bacc("TRN2", target_bir_lowering=False, debug=False, num_devices=M)
    zh_d = [
        nc.dram_tensor(f"zh{i}", [128, KPT, 128], f16, kind="ExternalInput").ap()
        for i in range(NZH)
    ]
    ypk_d = nc.dram_tensor("ypk", [128, KA], f32, kind="ExternalInput").ap()
    ybc_d = nc.dram_tensor("ybc", [128, B], f16, kind="ExternalInput").ap()
    zt_d = nc.dram_tensor("ztown", [D, BL], f16, kind="ExternalInput").ap()
    zown_d = nc.dram_tensor("zown", [128, KC, 128], f16, kind="ExternalInput").ap()
    iota_d = nc.dram_tensor("iota", [128, C], f16, kind="ExternalInput").ap()
    icol_d = nc.dram_tensor("icol", [128, KC], f32, kind="ExternalInput").ap()
    identh_d = nc.dram_tensor("identh", [128, 128], f16, kind="ExternalInput").ap()
    identf_d = nc.dram_tensor("identf", [128, 128], f32, kind="ExternalInput").ap()
    epseye_d = nc.dram_tensor("epseye", [128, 128], f32, kind="ExternalInput").ap()
    tceye_d = nc.dram_tensor("tceye", [128, 128], f32, kind="ExternalInput").ap()
    colsel_d = nc.dram_tensor("colsel", [128, KC * KC], f16, kind="ExternalInput").ap()
    rowsel_d = nc.dram_tensor("rowsel", [KC, C], f16, kind="ExternalInput").ap()
    out_d = nc.dram_tensor("scores", [BL, C], f32, kind="ExternalOutput").ap()

    with tile.TileContext(nc) as tc:
        _body(tc, out_d, zh_d, ypk_d, ybc_d, zt_d, zown_d, iota_d, icol_d,
              identh_d, identf_d, epseye_d, tceye_d, colsel_d, rowsel_d)
    nc.compile()
    return nc


def _body(tc, out_d, zh_d, ypk_d, ybc_d, zt_d, zown_d, iota_d, icol_d,
          identh_d, identf_d, epseye_d, tceye_d, colsel_d, rowsel_d):
    nc = tc.nc
    with (
        tc.tile_pool(name="const", bufs=1) as const,
        tc.tile_pool(name="io", bufs=1) as io,
        tc.tile_pool(name="sb", bufs=1) as sb,
        tc.tile_pool(name="small", bufs=1) as small,
    ):
        # ---- input DMAs (small constants first, then bulk) ----
        iota = const.tile([128, C], f16)
        nc.sync.dma_start(iota[:], iota_d)
        ypk = const.tile([128, KA], f32)
        nc.sync.dma_start(ypk[:], ypk_d)
        ident_h = const.tile([128, 128], f16)
        nc.sync.dma_start(ident_h[:], identh_d)
        ident_f = const.tile([128, 128], f32)
        nc.sync.dma_start(ident_f[:], identf_d)
        eps_eye = const.tile([128, 128], f32)
        nc.sync.dma_start(eps_eye[:], epseye_d)
        tc_eye = const.tile([128, 128], f32)
        nc.sync.dma_start(tc_eye[:], tceye_d)
        colsel = const.tile([128, KC * KC], f16)
        nc.sync.dma_start(colsel[:], colsel_d)
        rowsel = const.tile([KC, C], f16)
        nc.sync.dma_start(rowsel[:], rowsel_d)
        icol = const.tile([128, KC], f32)
        nc.sync.dma_start(icol[:], icol_d)

        zh = []
        for i in range(NZH):
            zt_i = io.tile([128, KPT, 128], f16, tag=f"zh{i}")
            nc.sync.dma_start(zt_i[:], zh_d[i])
            zh.append(zt_i)
        ybc = io.tile([128, B], f16)
        nc.sync.dma_start(ybc[:], ybc_d)
        ztown = io.tile([D, BL], f16)
        nc.sync.dma_start(ztown[:], zt_d)
        zown = io.tile([128, KC, 128], f16)
        nc.sync.dma_start(zown[:], zown_d)

        # preload the Ln activation table while everything else runs
        tbl = small.tile([1, 1], f32)
        nc.scalar.activation(tbl[:], ident_f[0:1, 0:1], AF.Ln)

        with tc.tile_pool(name="psStats", bufs=1, space="PSUM") as psS:
            ps_s1t = psS.tile([128, C], f32)
            ps_ztz = psS.tile([128, 128], f32)

            # ---- phase A: stats over all B rows ----
            for k in range(KA):
                zc = zh[k // KPT][:, k % KPT, :]
                oh = sb.tile([128, C], f16, tag="oh", bufs=4)
                nc.vector.tensor_scalar(
                    out=oh[:], in0=iota[:], scalar1=ypk[:, k : k + 1], scalar2=None,
                    op0=AL.is_equal,
                )
                st, sp = k == 0, k == KA - 1
                nc.tensor.matmul(ps_ztz[:], lhsT=zc, rhs=zc, start=st, stop=sp)
                nc.tensor.matmul(ps_s1t[:], lhsT=zc, rhs=oh[:], start=st, stop=sp)

            # counts via DVE free-dim reduction over [c_part, b] one-hot planes
            cnt4 = small.tile([128, KC], f32)
            for j in range(KC):
                plane = sb.tile([128, B], f16, tag="plane", bufs=1)
                nc.vector.tensor_scalar(
                    out=plane[:], in0=ybc[:], scalar1=icol[:, j : j + 1], scalar2=0.0,
                    op0=AL.is_equal, op1=AL.add, accum_out=cnt4[:, j : j + 1],
                )

            # ---- per-class scalar math [128, 4] ----
            cnts4 = small.tile([128, KC], f32)
            nc.vector.tensor_scalar(
                out=cnts4[:], in0=cnt4[:], scalar1=EPS, scalar2=None, op0=AL.add
            )
            rl_rc = small.tile([128, KC], f32)
            nc.vector.reciprocal(rl_rc[:], cnts4[:])
            rl_ln = small.tile([128, KC], f32)
            nc.scalar.activation(rl_ln[:], cnts4[:], AF.Ln)
            w4a = small.tile([128, KC], f32)
            nc.vector.tensor_scalar(
                out=w4a[:], in0=cnts4[:], scalar1=EPS, scalar2=None, op0=AL.add
            )
            w4b = small.tile([128, KC], f32)
            nc.vector.tensor_tensor(w4b[:], w4a[:], rl_rc[:], op=AL.mult)
            w4f = small.tile([128, KC], f32)
            nc.vector.tensor_tensor(w4f[:], w4b[:], rl_rc[:], op=AL.mult)

            with tc.tile_pool(name="psB", bufs=1, space="PSUM") as psB:
                # rcp and ln(counts) to [4, 128] class-chunk-partition layout
                ps_rc4 = psB.tile([KC, 128], f32)
                nc.tensor.transpose(ps_rc4[:], rl_rc[:], ident_f[:])
                ps_ln4 = psB.tile([KC, 128], f32)
                nc.tensor.transpose(ps_ln4[:], rl_ln[:], ident_f[:])
                rc4_h = small.tile([KC, 128], f16)
                nc.vector.tensor_scalar(
                    out=rc4_h[:], in0=ps_rc4[:], scalar1=CLAMP, scalar2=None, op0=AL.min
                )
                ln4_h = small.tile([KC, 128], f16)
                nc.vector.tensor_copy(ln4_h[:], ps_ln4[:])

                # rc broadcast [128, C] via PE outer products (row-select stationary)
                ps_rcb = psB.tile([128, C], f32)
                for j in range(KC):
                    nc.tensor.matmul(
                        ps_rcb[:, j * 128 : (j + 1) * 128],
                        lhsT=rowsel[:, j * 128 : (j + 1) * 128], rhs=rc4_h[:],
                        start=True, stop=True,
                    )

                # W2 = sum_c w_c S1_c S1_c^T (asymmetric: scale one side by w)
                s1_h = sb.tile([128, C], f16)
                nc.vector.tensor_copy(s1_h[:], ps_s1t[:])
                ztz_sb = sb.tile([128, 128], f32)
                nc.vector.tensor_copy(ztz_sb[:], ps_ztz[:])
                ps_w2 = psB.tile([128, 128], f32)
                for j in range(KC):
                    ps_tr = psB.tile([128, 128], f16, tag="tr", bufs=2)
                    nc.tensor.transpose(
                        ps_tr[:], s1_h[:, j * 128 : (j + 1) * 128], ident_h[:]
                    )
                    uj = sb.tile([128, 128], f16, tag="uj", bufs=2)
                    nc.vector.tensor_copy(uj[:], ps_tr[:])
                    vj = sb.tile([128, 128], f16, tag="vj", bufs=2)
                    nc.vector.tensor_scalar(
                        out=vj[:], in0=uj[:], scalar1=w4f[:, j : j + 1], scalar2=None,
                        op0=AL.mult,
                    )
                    nc.tensor.matmul(
                        ps_w2[:], lhsT=vj[:], rhs=uj[:], start=(j == 0), stop=(j == KC - 1)
                    )

                # mean^T in fp16 (meanT = S1T * (1/counts) column-broadcast)
                meanT = sb.tile([128, C], f16)
                nc.vector.tensor_tensor(meanT[:], s1_h[:], ps_rcb[:], op=AL.mult)

                # pooled covariance and Newton-Schulz X1 (both from pooled_f)
                pooled_f = sb.tile([128, 128], f32)
                nc.vector.tensor_tensor(pooled_f[:], ztz_sb[:], ps_w2[:], op=AL.subtract)
                pooled_h = sb.tile([128, 128], f16)
                nc.vector.scalar_tensor_tensor(
                    out=pooled_h[:], in0=pooled_f[:], scalar=1.0 / TOTAL,
                    in1=eps_eye[:], op0=AL.mult, op1=AL.add,
                )
                x_cur = sb.tile([128, 128], f16, tag="X", bufs=2)
                nc.vector.scalar_tensor_tensor(
                    out=x_cur[:], in0=pooled_f[:], scalar=-NS_C * NS_C / TOTAL,
                    in1=tc_eye[:], op0=AL.mult, op1=AL.add,
                )

        with tc.tile_pool(name="psNS", bufs=1, space="PSUM") as psN:
            for i in range(NS_ITERS):
                ps_t = psN.tile([128, 128], f32, tag="T", bufs=1)
                nc.tensor.matmul(ps_t[:], lhsT=pooled_h[:], rhs=x_cur[:], start=True, stop=True)
                t_h = sb.tile([128, 128], f16, tag="Th", bufs=2)
                nc.vector.tensor_copy(t_h[:], ps_t[:])
                ps_u = psN.tile([128, 128], f32, tag="U", bufs=1)
                nc.tensor.matmul(ps_u[:], lhsT=x_cur[:], rhs=t_h[:], start=True, stop=True)
                x_new = sb.tile([128, 128], f16, tag="X", bufs=2)
                nc.vector.scalar_tensor_tensor(
                    out=x_new[:], in0=x_cur[:], scalar=2.0, in1=ps_u[:],
                    op0=AL.mult, op1=AL.subtract,
                )
                x_cur = x_new

        # ---- phase C setup: q_b early (only needs P), then Pmt ----
        with (
            tc.tile_pool(name="psT1", bufs=1, space="PSUM") as psT1,
            tc.tile_pool(name="psC", bufs=1, space="PSUM") as psC,
        ):
            qms = []
            for k in range(KC):
                ps_zp = psC.tile([128, 128], f32, tag="zp", bufs=2)
                nc.tensor.matmul(
                    ps_zp[:], lhsT=ztown[:, k * 128 : (k + 1) * 128], rhs=x_cur[:],
                    start=True, stop=True,
                )
                zpz = sb.tile([128, 128], f16, tag="zpz", bufs=2)
                qm = small.tile([128, 1], f32, tag="qm", bufs=KC)
                nc.vector.scalar_tensor_tensor(
                    out=zpz[:], in0=ps_zp[:], scalar=-0.5, in1=zown[:, k, :],
                    op0=AL.mult, op1=AL.mult, accum_out=qm[:],
                )
                qms.append(qm)

            ps_pmt = psT1.tile([128, C], f32)
            nc.tensor.matmul(ps_pmt[:], lhsT=x_cur[:], rhs=meanT[:], start=True, stop=True)
            pmt_h = sb.tile([128, C], f16)
            nc.vector.tensor_copy(pmt_h[:], ps_pmt[:])

            # scores quad term for own rows can start as soon as pmt_h lands
            ps_gs = []
            for k in range(KC):
                ps_g = psC.tile([128, C], f32, tag="g", bufs=2)
                nc.tensor.matmul(
                    ps_g[:], lhsT=ztown[:, k * 128 : (k + 1) * 128], rhs=pmt_h[:],
                    start=True, stop=True,
                )
                ps_gs.append(ps_g)

            # r_c = colsum(0.5 * meanT . Pmt) into [4, 128] via col-select
            prod_h = sb.tile([128, C], f16)
            nc.vector.scalar_tensor_tensor(
                out=prod_h[:], in0=ps_pmt[:], scalar=0.5, in1=meanT[:],
                op0=AL.mult, op1=AL.mult,
            )
            ps_r4 = psT1.tile([KC, 128], f32)
            for j in range(KC):
                nc.tensor.matmul(
                    ps_r4[:], lhsT=colsel[:, j * KC : (j + 1) * KC],
                    rhs=prod_h[:, j * 128 : (j + 1) * 128],
                    start=(j == 0), stop=(j == KC - 1),
                )
            # rowcombo = ln(counts) - ln(total) - r   (r already halved)
            rc4 = small.tile([KC, 128], f16)
            nc.vector.scalar_tensor_tensor(
                out=rc4[:], in0=ln4_h[:], scalar=-LN_TOTAL, in1=ps_r4[:],
                op0=AL.add, op1=AL.subtract,
            )
            # broadcast to [128, C] via PE outers, then to SBUF for phase C
            ps_rc2 = psT1.tile([128, C], f32)
            for j in range(KC):
                nc.tensor.matmul(
                    ps_rc2[:, j * 128 : (j + 1) * 128],
                    lhsT=rowsel[:, j * 128 : (j + 1) * 128], rhs=rc4[:],
                    start=True, stop=True,
                )
            rc2_sb = sb.tile([128, C], f16)
            nc.vector.tensor_copy(rc2_sb[:], ps_rc2[:])

            # ---- phase C: final scores ----
            for k in range(KC):
                oc = sb.tile([128, C], f32, tag="oc", bufs=2)
                nc.vector.scalar_tensor_tensor(
                    out=oc[:], in0=ps_gs[k][:], scalar=qms[k][:], in1=rc2_sb[:],
                    op0=AL.add, op1=AL.add,
                )
                nc.sync.dma_start(out_d[k * 128 : (k + 1) * 128, :], oc[:])


_NC_CACHE = {}


def _get_nc():
    if "nc" not in _NC_CACHE:
        _NC_CACHE["nc"] = build_program()
    return _NC_CACHE["nc"]


def _consts():
    eye = np.eye(128, dtype=np.float32)
    iota = np.broadcast_to(np.arange(C, dtype=np.float16), (128, C))
    icol = (np.arange(128, dtype=np.float32)[:, None]
            + 128.0 * np.arange(KC, dtype=np.float32)[None, :]).astype(np.float32)
    colsel = np.zeros((128, KC * KC), dtype=np.float16)
    for j in range(KC):
        colsel[:, j * KC + j] = 1.0
    rowsel = np.zeros((KC, C), dtype=np.float16)
    for j in range(KC):
        rowsel[j, j * 128 : (j + 1) * 128] = 1.0
    return {
        "iota": np.ascontiguousarray(iota),
        "icol": icol,
        "identh": eye.astype(np.float16),
        "identf": eye,
        "epseye": (EPS * eye).astype(np.float32),
        "tceye": ((2.0 * NS_C - NS_C * NS_C * EPS) * eye).astype(np.float32),
        "colsel": colsel,
        "rowsel": rowsel,
    }


def make_in_maps(z, y):
    z = np.asarray(z, dtype=np.float32)
    y = np.asarray(y).astype(np.float32)
    zh = np.ascontiguousarray(
        z.reshape(KA, 128, 128).transpose(1, 0, 2).astype(np.float16)
    )
    y16 = y.astype(np.float16)
    ypk = np.ascontiguousarray(y.reshape(KA, 128).T.astype(np.float32))
    ybc = np.ascontiguousarray(np.broadcast_to(y16[None, :], (128, B)))
    consts = _consts()
    shared = {f"zh{i}": np.ascontiguousarray(zh[:, i * KPT : (i + 1) * KPT, :])
              for i in range(NZH)}
    shared.update({"ypk": ypk, "ybc": ybc})
    shared.update(consts)
    in_maps = []
    for m in range(M):
        zs = z[m * BL : (m + 1) * BL]
        zs16 = zs.astype(np.float16)
        d = dict(shared)
        d["ztown"] = np.ascontiguousarray(zs16.T)
        d["zown"] = np.ascontiguousarray(
            zs16.reshape(KC, 128, 128).transpose(1, 0, 2)
        )
        in_maps.append(d)
    return in_maps


def kernel(z, y):
    z = np.asarray(z)
    y = np.asarray(y)
    assert z.shape == (B, D) and y.shape == (B,)
    nc = _get_nc()
    in_maps = make_in_maps(z, y)
    res = run_bass_kernel_spmd(nc, in_maps, list(range(M)), trace=False)
    out = np.concatenate([res.results[m]["scores"] for m in range(M)], axis=0)
    return out.astype(np.float32)


if __name__ == "__main__":
    rng = np.random.default_rng(0)
    z = rng.standard_normal((B, D), dtype=np.float32)
    y = rng.integers(0, C, size=B).astype(np.int32)
    out = kernel(z, y)
    print("scores:", out.shape, out.dtype, out[:2, :4])


# revision 8
# speedup vs baseline: 2.0253x; 1.1976x over previous
"""LDA head forward on 8 Trainium2 NeuronCores (Bass/Tile).

Fully replicated statistics — ZERO collectives. Rationale: the SPMD launch
skews core start times by tens of us, so any cross-core barrier (AllReduce)
makes every core wait for the last-launched one. Instead each core
redundantly computes the full-batch statistics (phase A over all B=4096
rows) and only the [B_l, C] scoring (phase C) is sharded. No core ever
waits on another.

Per core:
  phase A: for each of 32 row-chunks: one-hot via iota+is_equal (fp16),
           PE accumulates S1T = Z^T OH [D,C] and ZtZ = Z^T Z [D,D] in PSUM.
           counts come from the DVE free-dim reduction (accum_out) over
           class-partition one-hot planes [c_part, b] — no PE stream needed.
  phase B: pooled = (ZtZ - sum_c w_c S1_c S1_c^T)/total + eps*I with
           w_c = (counts_c + eps)/counts_c^2 (asymmetric Gram: no sqrt),
           precision P via Newton-Schulz in fp16 (X1 = 2c I - c^2 A direct),
           Pmt = P @ (S1T diag(1/counts)).
  phase C: scores[b,c] = lnprior_c - 0.5 r_c - 0.5 q_b + (Z P mean^T)[b,c]
           on the core's own 512 rows.

fp16 is used for all PE streams (1 cycle/row vs 4 for fp32 and for tf32 on
<256-col outputs). All accumulation stays fp32 in PSUM. Per-class scalars
that can overflow fp16 (1/counts = 1e5 for an empty class) are clamped; the
clamped values only ever multiply exact zeros (S1_c = 0 for empty c).
Row/col broadcasts are PE outer products with one-hot selector stationaries
(no gpsimd partition_broadcast on the critical path).
"""

import numpy as np

import concourse.bacc as bacc
import concourse.mybir as mybir
import concourse.tile as tile
from concourse.bass_utils import run_bass_kernel_spmd

f32 = mybir.dt.float32
f16 = mybir.dt.float16
AL = mybir.AluOpType
AF = mybir.ActivationFunctionType

M = 8            # cores
B = 4096
D = 128
C = 512
BL = B // M      # 512 rows per core
KC = BL // 128   # 4 own chunks of 128 rows
KA = B // 128    # 32 total chunks
EPS = 1e-5
TOTAL = float(B) + C * EPS
LN_TOTAL = float(np.log(np.float64(TOTAL)))
NS_C = 1.05      # Newton-Schulz init scale; X1 = 2c*I - c^2*A
NS_ITERS = 2
CLAMP = 60000.0  # fp16-safe cap for per-class reciprocals/weights
NZH = 8          # z arrives in NZH separate DMA tiles so PE can start early
KPT = KA // NZH  # chunks per z tile


def build_program():
    nc = bacc.Bacc("TRN2", target_bir_lowering=False, debug=False, num_devices=M)
    zh_d = [
        nc.dram_tensor(f"zh{i}", [128, KPT, 128], f16, kind="ExternalInput").ap()
        for i in range(NZH)
    ]
    ypk_d = nc.dram_tensor("ypk", [128, KA], f32, kind="ExternalInput").ap()
    ybc_d = nc.dram_tensor("ybc", [128, B], f16, kind="ExternalInput").ap()
    zt_d = nc.dram_tensor("ztown", [D, BL], f16, kind="ExternalInput").ap()
    zown_d = nc.dram_tensor("zown", [128, KC, 128], f16, kind="ExternalInput").ap()
    iota_d = nc.dram_tensor("iota", [128, C], f16, kind="ExternalInput").ap()
    icol_d = nc.dram_tensor("icol", [128, KC], f32, kind="ExternalInput").ap()
    identh_d = nc.dram_tensor("identh", [128, 128], f16, kind="ExternalInput").ap()
    identf_d = nc.dram_tensor("identf", [128, 128], f32, kind="ExternalInput").ap()
    epseye_d = nc.dram_tensor("epseye", [128, 128], f32, kind="ExternalInput").ap()
    tceye_d = nc.dram_tensor("tceye", [128, 128], f32, kind="ExternalInput").ap()
    colsel_d = nc.dram_tensor("colsel", [128, KC * KC], f16, kind="ExternalInput").ap()
    rowsel_d = nc.dram_tensor("rowsel", [KC, C], f16, kind="ExternalInput").ap()
    out_d = nc.dram_tensor("scores", [BL, C], f32, kind="ExternalOutput").ap()

    with tile.TileContext(nc) as tc:
        _body(tc, out_d, zh_d, ypk_d, ybc_d, zt_d, zown_d, iota_d, icol_d,
              identh_d, identf_d, epseye_d, tceye_d, colsel_d, rowsel_d)
    nc.compile()
    return nc


def _body(tc, out_d, zh_d, ypk_d, ybc_d, zt_d, zown_d, iota_d, icol_d,
          identh_d, identf_d, epseye_d, tceye_d, colsel_d, rowsel_d):
    nc = tc.nc
    with (
        tc.tile_pool(name="const", bufs=1) as const,
        tc.tile_pool(name="io", bufs=1) as io,
        tc.tile_pool(name="sb", bufs=1) as sb,
        tc.tile_pool(name="small", bufs=1) as small,
    ):
        # ---- input DMAs: oh deps + bulk z first, tail-phase constants later ----
        iota = const.tile([128, C], f16)
        nc.sync.dma_start(iota[:], iota_d)
        ypk = const.tile([128, KA], f32)
        nc.sync.dma_start(ypk[:], ypk_d)
        icol = const.tile([128, KC], f32)
        nc.sync.dma_start(icol[:], icol_d)
        zh = []
        for i in range(NZH):
            zt_i = io.tile([128, KPT, 128], f16, tag=f"zh{i}")
            nc.sync.dma_start(zt_i[:], zh_d[i])
            zh.append(zt_i)
        ybc = io.tile([128, B], f16)
        nc.sync.dma_start(ybc[:], ybc_d)
        ident_h = const.tile([128, 128], f16)
        nc.sync.dma_start(ident_h[:], identh_d)
        ident_f = const.tile([128, 128], f32)
        nc.sync.dma_start(ident_f[:], identf_d)
        eps_eye = const.tile([128, 128], f32)
        nc.sync.dma_start(eps_eye[:], epseye_d)
        tc_eye = const.tile([128, 128], f32)
        nc.sync.dma_start(tc_eye[:], tceye_d)
        colsel = const.tile([128, KC * KC], f16)
        nc.sync.dma_start(colsel[:], colsel_d)
        rowsel = const.tile([KC, C], f16)
        nc.sync.dma_start(rowsel[:], rowsel_d)
        ztown = io.tile([D, BL], f16)
        nc.sync.dma_start(ztown[:], zt_d)
        zown = io.tile([128, KC, 128], f16)
        nc.sync.dma_start(zown[:], zown_d)

        # preload the Ln activation table while everything else runs
        tbl = small.tile([1, 1], f32)
        nc.scalar.activation(tbl[:], ident_f[0:1, 0:1], AF.Ln)

        with tc.tile_pool(name="psStats", bufs=1, space="PSUM") as psS:
            ps_s1t = psS.tile([128, C], f32)
            ps_ztz = psS.tile([128, 128], f32)

            # ---- phase A: stats over all B rows; counts planes interleaved ----
            # planes (counts) are interleaved into the oh stream so the DVE
            # fills PE-paced gaps instead of serializing after phase A.
            cnt4h = small.tile([128, KC], f16)
            plane_after = {12: 0, 17: 1, 22: 2, 27: 3}
            for k in range(KA):
                zc = zh[k // KPT][:, k % KPT, :]
                oh = sb.tile([128, C], f16, tag="oh", bufs=6)
                nc.vector.tensor_scalar(
                    out=oh[:], in0=iota[:], scalar1=ypk[:, k : k + 1], scalar2=None,
                    op0=AL.is_equal,
                )
                st, sp = k == 0, k == KA - 1
                nc.tensor.matmul(ps_ztz[:], lhsT=zc, rhs=zc, start=st, stop=sp)
                nc.tensor.matmul(ps_s1t[:], lhsT=zc, rhs=oh[:], start=st, stop=sp)
                if k in plane_after:
                    j = plane_after[k]
                    plane = sb.tile([128, B], f16, tag="plane", bufs=1)
                    nc.vector.tensor_scalar(
                        out=plane[:], in0=ybc[:], scalar1=icol[:, j : j + 1],
                        scalar2=0.0, op0=AL.is_equal, op1=AL.add,
                        accum_out=cnt4h[:, j : j + 1],
                    )
            cnt4 = small.tile([128, KC], f32)
            nc.gpsimd.tensor_copy(cnt4[:], cnt4h[:])

            # ---- per-class scalar math [128, 4] ----
            cnts4 = small.tile([128, KC], f32)
            nc.vector.tensor_scalar(
                out=cnts4[:], in0=cnt4[:], scalar1=EPS, scalar2=None, op0=AL.add
            )
            rl_rc = small.tile([128, KC], f32)
            nc.vector.reciprocal(rl_rc[:], cnts4[:])
            rl_ln = small.tile([128, KC], f32)
            nc.scalar.activation(rl_ln[:], cnts4[:], AF.Ln)
            w4a = small.tile([128, KC], f32)
            nc.vector.tensor_scalar(
                out=w4a[:], in0=cnts4[:], scalar1=EPS, scalar2=None, op0=AL.add
            )
            w4b = small.tile([128, KC], f32)
            nc.vector.tensor_tensor(w4b[:], w4a[:], rl_rc[:], op=AL.mult)
            w4f = small.tile([128, KC], f32)
            nc.vector.tensor_tensor(w4f[:], w4b[:], rl_rc[:], op=AL.mult)

            with tc.tile_pool(name="psB", bufs=1, space="PSUM") as psB:
                # rcp and ln(counts) to [4, 128] class-chunk-partition layout
                ps_rc4 = psB.tile([KC, 128], f32)
                nc.tensor.transpose(ps_rc4[:], rl_rc[:], ident_f[:])
                ps_ln4 = psB.tile([KC, 128], f32)
                nc.tensor.transpose(ps_ln4[:], rl_ln[:], ident_f[:])
                rc4_h = small.tile([KC, 128], f16)
                nc.vector.tensor_scalar(
                    out=rc4_h[:], in0=ps_rc4[:], scalar1=CLAMP, scalar2=None, op0=AL.min
                )
                ln4_h = small.tile([KC, 128], f16)
                nc.scalar.copy(ln4_h[:], ps_ln4[:])

                # rc broadcast [128, C] via PE outer products (row-select stationary)
                ps_rcb = psB.tile([128, C], f32)
                for j in range(KC):
                    nc.tensor.matmul(
                        ps_rcb[:, j * 128 : (j + 1) * 128],
                        lhsT=rowsel[:, j * 128 : (j + 1) * 128], rhs=rc4_h[:],
                        start=True, stop=True,
                    )

                # W2 = sum_c w_c S1_c S1_c^T (asymmetric: scale one side by w)
                s1_h = sb.tile([128, C], f16)
                nc.vector.tensor_copy(s1_h[:], ps_s1t[:])
                ztz_sb = sb.tile([128, 128], f32)
                nc.vector.tensor_copy(ztz_sb[:], ps_ztz[:])
                ps_w2 = psB.tile([128, 128], f32)
                for j in range(KC):
                    ps_tr = psB.tile([128, 128], f16, tag="tr", bufs=2)
                    nc.tensor.transpose(
                        ps_tr[:], s1_h[:, j * 128 : (j + 1) * 128], ident_h[:]
                    )
                    uj = sb.tile([128, 128], f16, tag="uj", bufs=2)
                    nc.scalar.copy(uj[:], ps_tr[:])
                    vj = sb.tile([128, 128], f16, tag="vj", bufs=2)
                    nc.vector.tensor_scalar(
                        out=vj[:], in0=uj[:], scalar1=w4f[:, j : j + 1], scalar2=None,
                        op0=AL.mult,
                    )
                    nc.tensor.matmul(
                        ps_w2[:], lhsT=vj[:], rhs=uj[:], start=(j == 0), stop=(j == KC - 1)
                    )

                # mean^T in fp16 (meanT = S1T * (1/counts) column-broadcast)
                meanT = sb.tile([128, C], f16)
                nc.vector.tensor_tensor(meanT[:], s1_h[:], ps_rcb[:], op=AL.mult)

                # pooled covariance and Newton-Schulz X1 (both from pooled_f)
                pooled_f = sb.tile([128, 128], f32)
                nc.vector.tensor_tensor(pooled_f[:], ztz_sb[:], ps_w2[:], op=AL.subtract)
                pooled_h = sb.tile([128, 128], f16)
                nc.vector.scalar_tensor_tensor(
                    out=pooled_h[:], in0=pooled_f[:], scalar=1.0 / TOTAL,
                    in1=eps_eye[:], op0=AL.mult, op1=AL.add,
                )
                x_cur = sb.tile([128, 128], f16, tag="X", bufs=2)
                nc.vector.scalar_tensor_tensor(
                    out=x_cur[:], in0=pooled_f[:], scalar=-NS_C * NS_C / TOTAL,
                    in1=tc_eye[:], op0=AL.mult, op1=AL.add,
                )

        with tc.tile_pool(name="psNS", bufs=1, space="PSUM") as psN:
            for i in range(NS_ITERS):
                ps_t = psN.tile([128, 128], f32, tag="T", bufs=1)
                nc.tensor.matmul(ps_t[:], lhsT=pooled_h[:], rhs=x_cur[:], start=True, stop=True)
                t_h = sb.tile([128, 128], f16, tag="Th", bufs=2)
                nc.scalar.copy(t_h[:], ps_t[:])
                ps_u = psN.tile([128, 128], f32, tag="U", bufs=1)
                nc.tensor.matmul(ps_u[:], lhsT=x_cur[:], rhs=t_h[:], start=True, stop=True)
                x_new = sb.tile([128, 128], f16, tag="X", bufs=2)
                nc.vector.scalar_tensor_tensor(
                    out=x_new[:], in0=x_cur[:], scalar=2.0, in1=ps_u[:],
                    op0=AL.mult, op1=AL.subtract,
                )
                x_cur = x_new

        # ---- phase C setup: q_b early (only needs P), then Pmt ----
        with (
            tc.tile_pool(name="psT1", bufs=1, space="PSUM") as psT1,
            tc.tile_pool(name="psC", bufs=1, space="PSUM") as psC,
        ):
            qms = []
            for k in range(KC):
                ps_zp = psC.tile([128, 128], f32, tag="zp", bufs=2)
                nc.tensor.matmul(
                    ps_zp[:], lhsT=ztown[:, k * 128 : (k + 1) * 128], rhs=x_cur[:],
                    start=True, stop=True,
                )
                zpz = sb.tile([128, 128], f16, tag="zpz", bufs=2)
                qm = small.tile([128, 1], f32, tag="qm", bufs=KC)
                nc.vector.scalar_tensor_tensor(
                    out=zpz[:], in0=ps_zp[:], scalar=-0.5, in1=zown[:, k, :],
                    op0=AL.mult, op1=AL.mult, accum_out=qm[:],
                )
                qms.append(qm)

            ps_pmt = psT1.tile([128, C], f32)
            nc.tensor.matmul(ps_pmt[:], lhsT=x_cur[:], rhs=meanT[:], start=True, stop=True)
            pmt_h = sb.tile([128, C], f16)
            nc.scalar.copy(pmt_h[:], ps_pmt[:])

            # scores quad term for own rows can start as soon as pmt_h lands
            ps_gs = []
            for k in range(KC):
                ps_g = psC.tile([128, C], f32, tag="g", bufs=3)
                nc.tensor.matmul(
                    ps_g[:], lhsT=ztown[:, k * 128 : (k + 1) * 128], rhs=pmt_h[:],
                    start=True, stop=True,
                )
                ps_gs.append(ps_g)

            # r_c = colsum(0.5 * meanT . Pmt) into [4, 128] via col-select
            prod_h = sb.tile([128, C], f16)
            nc.vector.scalar_tensor_tensor(
                out=prod_h[:], in0=ps_pmt[:], scalar=0.5, in1=meanT[:],
                op0=AL.mult, op1=AL.mult,
            )
            ps_r4 = psT1.tile([KC, 128], f32)
            for j in range(KC):
                nc.tensor.matmul(
                    ps_r4[:], lhsT=colsel[:, j * KC : (j + 1) * KC],
                    rhs=prod_h[:, j * 128 : (j + 1) * 128],
                    start=(j == 0), stop=(j == KC - 1),
                )
            # rowcombo = ln(counts) - ln(total) - r   (r already halved)
            rc4 = small.tile([KC, 128], f16)
            nc.vector.scalar_tensor_tensor(
                out=rc4[:], in0=ln4_h[:], scalar=-LN_TOTAL, in1=ps_r4[:],
                op0=AL.add, op1=AL.subtract,
            )
            # broadcast to [128, C] via PE outers, then to SBUF for phase C
            ps_rc2 = psT1.tile([128, C], f32)
            for j in range(KC):
                nc.tensor.matmul(
                    ps_rc2[:, j * 128 : (j + 1) * 128],
                    lhsT=rowsel[:, j * 128 : (j + 1) * 128], rhs=rc4[:],
                    start=True, stop=True,
                )
            rc2_sb = sb.tile([128, C], f16)
            nc.scalar.copy(rc2_sb[:], ps_rc2[:])

            # ---- phase C: final scores ----
            for k in range(KC):
                oc = sb.tile([128, C], f32, tag="oc", bufs=2)
                nc.vector.scalar_tensor_tensor(
                    out=oc[:], in0=ps_gs[k][:], scalar=qms[k][:], in1=rc2_sb[:],
                    op0=AL.add, op1=AL.add,
                )
                nc.sync.dma_start(out_d[k * 128 : (k + 1) * 128, :], oc[:])


_NC_CACHE = {}


def _get_nc():
    if "nc" not in _NC_CACHE:
        _NC_CACHE["nc"] = build_program()
    return _NC_CACHE["nc"]


def _consts():
    eye = np.eye(128, dtype=np.float32)
    iota = np.broadcast_to(np.arange(C, dtype=np.float16), (128, C))
    icol = (np.arange(128, dtype=np.float32)[:, None]
            + 128.0 * np.arange(KC, dtype=np.float32)[None, :]).astype(np.float32)
    colsel = np.zeros((128, KC * KC), dtype=np.float16)
    for j in range(KC):
        colsel[:, j * KC + j] = 1.0
    rowsel = np.zeros((KC, C), dtype=np.float16)
    for j in range(KC):
        rowsel[j, j * 128 : (j + 1) * 128] = 1.0
    return {
        "iota": np.ascontiguousarray(iota),
        "icol": icol,
        "identh": eye.astype(np.float16),
        "identf": eye,
        "epseye": (EPS * eye).astype(np.float32),
        "tceye": ((2.0 * NS_C - NS_C * NS_C * EPS) * eye).astype(np.float32),
        "colsel": colsel,
        "rowsel": rowsel,
    }


def make_in_maps(z, y):
    z = np.asarray(z, dtype=np.float32)
    y = np.asarray(y).astype(np.float32)
    zh = np.ascontiguousarray(
        z.reshape(KA, 128, 128).transpose(1, 0, 2).astype(np.float16)
    )
    y16 = y.astype(np.float16)
    ypk = np.ascontiguousarray(y.reshape(KA, 128).T.astype(np.float32))
    ybc = np.ascontiguousarray(np.broadcast_to(y16[None, :], (128, B)))
    consts = _consts()
    shared = {f"zh{i}": np.ascontiguousarray(zh[:, i * KPT : (i + 1) * KPT, :])
              for i in range(NZH)}
    shared.update({"ypk": ypk, "ybc": ybc})
    shared.update(consts)
    in_maps = []
    for m in range(M):
        zs = z[m * BL : (m + 1) * BL]
        zs16 = zs.astype(np.float16)
        d = dict(shared)
        d["ztown"] = np.ascontiguousarray(zs16.T)
        d["zown"] = np.ascontiguousarray(
            zs16.reshape(KC, 128, 128).transpose(1, 0, 2)
        )
        in_maps.append(d)
    return in_maps


def kernel(z, y):
    z = np.asarray(z)
    y = np.asarray(y)
    assert z.shape == (B, D) and y.shape == (B,)
    nc = _get_nc()
    in_maps = make_in_maps(z, y)
    res = run_bass_kernel_spmd(nc, in_maps, list(range(M)), trace=False)
    out = np.concatenate([res.results[m]["scores"] for m in range(M)], axis=0)
    return out.astype(np.float32)


if __name__ == "__main__":
    rng = np.random.default_rng(0)
    z = rng.standard_normal((B, D), dtype=np.float32)
    y = rng.integers(0, C, size=B).astype(np.int32)
    out = kernel(z, y)
    print("scores:", out.shape, out.dtype, out[:2, :4])


# revision 11
# speedup vs baseline: 2.1330x; 1.0532x over previous
"""LDA head forward on 8 Trainium2 NeuronCores (Bass/Tile).

Fully replicated statistics — ZERO collectives. Rationale: the SPMD launch
skews core start times by tens of us, so any cross-core barrier (AllReduce)
makes every core wait for the last-launched one. Instead each core
redundantly computes the full-batch statistics (phase A over all B=4096
rows) and only the [B_l, C] scoring (phase C) is sharded. No core ever
waits on another.

Per core:
  phase A: for each of 32 row-chunks: one-hot via iota+is_equal (fp16),
           PE accumulates S1T = Z^T OH [D,C] and ZtZ = Z^T Z [D,D] in PSUM.
           counts come from the DVE free-dim reduction (accum_out) over
           class-partition one-hot planes [c_part, b] — no PE stream needed.
  phase B: pooled = (ZtZ - sum_c w_c S1_c S1_c^T)/total + eps*I with
           w_c = (counts_c + eps)/counts_c^2 (asymmetric Gram: no sqrt),
           precision P via Newton-Schulz in fp16 (X1 = 2c I - c^2 A direct),
           Pmt = P @ (S1T diag(1/counts)).
  phase C: scores[b,c] = lnprior_c - 0.5 r_c - 0.5 q_b + (Z P mean^T)[b,c]
           on the core's own 512 rows.

fp16 is used for all PE streams (1 cycle/row vs 4 for fp32 and for tf32 on
<256-col outputs). All accumulation stays fp32 in PSUM. Per-class scalars
that can overflow fp16 (1/counts = 1e5 for an empty class) are clamped; the
clamped values only ever multiply exact zeros (S1_c = 0 for empty c).
Row/col broadcasts are PE outer products with one-hot selector stationaries
(no gpsimd partition_broadcast on the critical path).
"""

import numpy as np

import concourse.bacc as bacc
import concourse.mybir as mybir
import concourse.tile as tile
from concourse.bass_utils import run_bass_kernel_spmd

f32 = mybir.dt.float32
f16 = mybir.dt.float16
AL = mybir.AluOpType
AF = mybir.ActivationFunctionType

M = 8            # cores
B = 4096
D = 128
C = 512
BL = B // M      # 512 rows per core
KC = BL // 128   # 4 own chunks of 128 rows
KA = B // 128    # 32 total chunks
EPS = 1e-5
TOTAL = float(B) + C * EPS
LN_TOTAL = float(np.log(np.float64(TOTAL)))
NS_C = 1.05      # Newton-Schulz init scale; X1 = 2c*I - c^2*A
NS_ITERS = 2
CLAMP = 60000.0  # fp16-safe cap for per-class reciprocals/weights
NZH = 8          # z arrives in NZH separate DMA tiles so PE can start early
KPT = KA // NZH  # chunks per z tile


def build_program():
    nc = bacc.Bacc("TRN2", target_bir_lowering=False, debug=False, num_devices=M)
    zh_d = [
        nc.dram_tensor(f"zh{i}", [128, KPT, 128], f16, kind="ExternalInput").ap()
        for i in range(NZH)
    ]
    ypk_d = nc.dram_tensor("ypk", [128, KA], f32, kind="ExternalInput").ap()
    ybc_d = nc.dram_tensor("ybc", [128, B], f16, kind="ExternalInput").ap()
    zt_d = nc.dram_tensor("ztown", [D, BL], f16, kind="ExternalInput").ap()
    zown_d = nc.dram_tensor("zown", [128, KC, 128], f16, kind="ExternalInput").ap()
    iota_d = nc.dram_tensor("iota", [128, C], f16, kind="ExternalInput").ap()
    icol_d = nc.dram_tensor("icol", [128, KC], f32, kind="ExternalInput").ap()
    identh_d = nc.dram_tensor("identh", [128, 128], f16, kind="ExternalInput").ap()
    identf_d = nc.dram_tensor("identf", [128, 128], f32, kind="ExternalInput").ap()
    epseye_d = nc.dram_tensor("epseye", [128, 128], f32, kind="ExternalInput").ap()
    tceye_d = nc.dram_tensor("tceye", [128, 128], f32, kind="ExternalInput").ap()
    colsel_d = nc.dram_tensor("colsel", [128, KC * KC], f16, kind="ExternalInput").ap()
    rowsel_d = nc.dram_tensor("rowsel", [KC, C], f16, kind="ExternalInput").ap()
    out_d = nc.dram_tensor("scores", [BL, C], f32, kind="ExternalOutput").ap()

    with tile.TileContext(nc) as tc:
        _body(tc, out_d, zh_d, ypk_d, ybc_d, zt_d, zown_d, iota_d, icol_d,
              identh_d, identf_d, epseye_d, tceye_d, colsel_d, rowsel_d)
    nc.compile()
    return nc


def _body(tc, out_d, zh_d, ypk_d, ybc_d, zt_d, zown_d, iota_d, icol_d,
          identh_d, identf_d, epseye_d, tceye_d, colsel_d, rowsel_d):
    nc = tc.nc
    with (
        tc.tile_pool(name="const", bufs=1) as const,
        tc.tile_pool(name="io", bufs=1) as io,
        tc.tile_pool(name="sb", bufs=1) as sb,
        tc.tile_pool(name="small", bufs=1) as small,
    ):
        # ---- input DMAs: oh deps + bulk z first, tail-phase constants later ----
        iota = const.tile([128, C], f16)
        nc.sync.dma_start(iota[:], iota_d)
        ypk = const.tile([128, KA], f32)
        nc.sync.dma_start(ypk[:], ypk_d)
        icol = const.tile([128, KC], f32)
        nc.sync.dma_start(icol[:], icol_d)
        zh = []
        for i in range(NZH):
            zt_i = io.tile([128, KPT, 128], f16, tag=f"zh{i}")
            nc.sync.dma_start(zt_i[:], zh_d[i])
            zh.append(zt_i)
        ybc = io.tile([128, B], f16)
        nc.sync.dma_start(ybc[:], ybc_d)
        ident_h = const.tile([128, 128], f16)
        nc.sync.dma_start(ident_h[:], identh_d)
        ident_f = const.tile([128, 128], f32)
        nc.sync.dma_start(ident_f[:], identf_d)
        eps_eye = const.tile([128, 128], f32)
        nc.sync.dma_start(eps_eye[:], epseye_d)
        tc_eye = const.tile([128, 128], f32)
        nc.sync.dma_start(tc_eye[:], tceye_d)
        colsel = const.tile([128, KC * KC], f16)
        nc.sync.dma_start(colsel[:], colsel_d)
        rowsel = const.tile([KC, C], f16)
        nc.sync.dma_start(rowsel[:], rowsel_d)
        ztown = io.tile([D, BL], f16)
        nc.sync.dma_start(ztown[:], zt_d)
        zown = io.tile([128, KC, 128], f16)
        nc.sync.dma_start(zown[:], zown_d)

        # preload the Ln activation table while everything else runs
        tbl = small.tile([1, 1], f32)
        nc.scalar.activation(tbl[:], ident_f[0:1, 0:1], AF.Ln)

        with tc.tile_pool(name="psStats", bufs=1, space="PSUM") as psS:
            ps_s1t = psS.tile([128, C], f32)
            ps_ztz = psS.tile([128, 128], f32)

            # ---- phase A: stats over all B rows; counts planes interleaved ----
            # planes (counts) are interleaved into the oh stream so the DVE
            # fills PE-paced gaps instead of serializing after phase A.
            cnt4h = small.tile([128, KC], f16)
            plane_after = {6: 0, 13: 1, 20: 2, 27: 3}
            for k in range(KA):
                zc = zh[k // KPT][:, k % KPT, :]
                oh = sb.tile([128, C], f16, tag="oh", bufs=6)
                nc.vector.tensor_scalar(
                    out=oh[:], in0=iota[:], scalar1=ypk[:, k : k + 1], scalar2=None,
                    op0=AL.is_equal,
                )
                st, sp = k == 0, k == KA - 1
                nc.tensor.matmul(ps_ztz[:], lhsT=zc, rhs=zc, start=st, stop=sp)
                nc.tensor.matmul(ps_s1t[:], lhsT=zc, rhs=oh[:], start=st, stop=sp)
                if k in plane_after:
                    j = plane_after[k]
                    plane = sb.tile([128, B], f16, tag="plane", bufs=1)
                    nc.vector.tensor_scalar(
                        out=plane[:], in0=ybc[:], scalar1=icol[:, j : j + 1],
                        scalar2=0.0, op0=AL.is_equal, op1=AL.add,
                        accum_out=cnt4h[:, j : j + 1],
                    )
            cnt4 = small.tile([128, KC], f32)
            nc.gpsimd.tensor_copy(cnt4[:], cnt4h[:])

            # ---- per-class scalar math [128, 4] ----
            cnts4 = small.tile([128, KC], f32)
            nc.vector.tensor_scalar(
                out=cnts4[:], in0=cnt4[:], scalar1=EPS, scalar2=None, op0=AL.add
            )
            rl_rc = small.tile([128, KC], f32)
            nc.vector.reciprocal(rl_rc[:], cnts4[:])
            rl_ln = small.tile([128, KC], f32)
            nc.scalar.activation(rl_ln[:], cnts4[:], AF.Ln)
            w4a = small.tile([128, KC], f32)
            nc.vector.tensor_scalar(
                out=w4a[:], in0=cnts4[:], scalar1=EPS, scalar2=None, op0=AL.add
            )
            w4b = small.tile([128, KC], f32)
            nc.vector.tensor_tensor(w4b[:], w4a[:], rl_rc[:], op=AL.mult)
            w4f = small.tile([128, KC], f32)
            nc.vector.tensor_tensor(w4f[:], w4b[:], rl_rc[:], op=AL.mult)

            with tc.tile_pool(name="psB", bufs=1, space="PSUM") as psB:
                # rcp and ln(counts) to [4, 128] class-chunk-partition layout
                ps_rc4 = psB.tile([KC, 128], f32)
                nc.tensor.transpose(ps_rc4[:], rl_rc[:], ident_f[:])
                ps_ln4 = psB.tile([KC, 128], f32)
                nc.tensor.transpose(ps_ln4[:], rl_ln[:], ident_f[:])
                rc4_h = small.tile([KC, 128], f16)
                nc.vector.tensor_scalar(
                    out=rc4_h[:], in0=ps_rc4[:], scalar1=CLAMP, scalar2=None, op0=AL.min
                )
                ln4_h = small.tile([KC, 128], f16)
                nc.scalar.copy(ln4_h[:], ps_ln4[:])

                # rc broadcast [128, C] via PE outer products (row-select stationary)
                ps_rcb = psB.tile([128, C], f32)
                for j in range(KC):
                    nc.tensor.matmul(
                        ps_rcb[:, j * 128 : (j + 1) * 128],
                        lhsT=rowsel[:, j * 128 : (j + 1) * 128], rhs=rc4_h[:],
                        start=True, stop=True,
                    )

                # W2 = sum_c w_c S1_c S1_c^T (asymmetric: scale one side by w)
                s1_h = sb.tile([128, C], f16)
                nc.vector.tensor_copy(s1_h[:], ps_s1t[:])
                ztz_sb = sb.tile([128, 128], f32)
                nc.vector.tensor_copy(ztz_sb[:], ps_ztz[:])
                ps_w2 = psB.tile([128, 128], f32)
                for j in range(KC):
                    ps_tr = psB.tile([128, 128], f16, tag="tr", bufs=2)
                    nc.tensor.transpose(
                        ps_tr[:], s1_h[:, j * 128 : (j + 1) * 128], ident_h[:]
                    )
                    uj = sb.tile([128, 128], f16, tag="uj", bufs=2)
                    nc.scalar.copy(uj[:], ps_tr[:])
                    vj = sb.tile([128, 128], f16, tag="vj", bufs=2)
                    nc.vector.tensor_scalar(
                        out=vj[:], in0=ps_tr[:], scalar1=w4f[:, j : j + 1], scalar2=None,
                        op0=AL.mult,
                    )
                    nc.tensor.matmul(
                        ps_w2[:], lhsT=vj[:], rhs=uj[:], start=(j == 0), stop=(j == KC - 1)
                    )

                # mean^T in fp16 (meanT = S1T * (1/counts) column-broadcast)
                meanT = sb.tile([128, C], f16)
                nc.vector.tensor_tensor(meanT[:], s1_h[:], ps_rcb[:], op=AL.mult)

                # pooled covariance and Newton-Schulz X1 (both from pooled_f)
                pooled_f = sb.tile([128, 128], f32)
                nc.vector.tensor_tensor(pooled_f[:], ztz_sb[:], ps_w2[:], op=AL.subtract)
                pooled_h = sb.tile([128, 128], f16)
                nc.vector.scalar_tensor_tensor(
                    out=pooled_h[:], in0=pooled_f[:], scalar=1.0 / TOTAL,
                    in1=eps_eye[:], op0=AL.mult, op1=AL.add,
                )
                x_cur = sb.tile([128, 128], f16, tag="X", bufs=2)
                nc.vector.scalar_tensor_tensor(
                    out=x_cur[:], in0=pooled_f[:], scalar=-1.0 / TOTAL,
                    in1=tc_eye[:], op0=AL.mult, op1=AL.add,
                )

        with tc.tile_pool(name="psNS", bufs=1, space="PSUM") as psN:
            for i in range(NS_ITERS):
                ps_t = psN.tile([128, 128], f32, tag="T", bufs=1)
                nc.tensor.matmul(ps_t[:], lhsT=pooled_h[:], rhs=x_cur[:], start=True, stop=True)
                t_h = sb.tile([128, 128], f16, tag="Th", bufs=2)
                nc.scalar.copy(t_h[:], ps_t[:])
                ps_u = psN.tile([128, 128], f32, tag="U", bufs=1)
                nc.tensor.matmul(ps_u[:], lhsT=x_cur[:], rhs=t_h[:], start=True, stop=True)
                x_new = sb.tile([128, 128], f16, tag="X", bufs=2)
                nc.vector.scalar_tensor_tensor(
                    out=x_new[:], in0=x_cur[:], scalar=2.0, in1=ps_u[:],
                    op0=AL.mult, op1=AL.subtract,
                )
                x_cur = x_new

        # ---- phase C setup: q_b early (only needs P), then Pmt ----
        with (
            tc.tile_pool(name="psT1", bufs=1, space="PSUM") as psT1,
            tc.tile_pool(name="psC", bufs=1, space="PSUM") as psC,
        ):
            qms = []
            for k in range(KC):
                ps_zp = psC.tile([128, 128], f32, tag="zp", bufs=2)
                nc.tensor.matmul(
                    ps_zp[:], lhsT=ztown[:, k * 128 : (k + 1) * 128], rhs=x_cur[:],
                    start=True, stop=True,
                )
                zpz = sb.tile([128, 128], f16, tag="zpz", bufs=2)
                qm = small.tile([128, 1], f32, tag="qm", bufs=KC)
                nc.vector.scalar_tensor_tensor(
                    out=zpz[:], in0=ps_zp[:], scalar=-0.5, in1=zown[:, k, :],
                    op0=AL.mult, op1=AL.mult, accum_out=qm[:],
                )
                qms.append(qm)

            ps_pmt = psT1.tile([128, C], f32)
            nc.tensor.matmul(ps_pmt[:], lhsT=x_cur[:], rhs=meanT[:], start=True, stop=True)
            pmt_h = sb.tile([128, C], f16)
            nc.scalar.copy(pmt_h[:], ps_pmt[:])
            # r_c = colsum(0.5 * meanT . Pmt) into [4, 128] via col-select
            prod_h = sb.tile([128, C], f16)
            nc.vector.scalar_tensor_tensor(
                out=prod_h[:], in0=ps_pmt[:], scalar=0.5, in1=meanT[:],
                op0=AL.mult, op1=AL.mult,
            )
            ps_r4 = psT1.tile([KC, 128], f32)
            for j in range(KC):
                nc.tensor.matmul(
                    ps_r4[:], lhsT=colsel[:, j * KC : (j + 1) * KC],
                    rhs=prod_h[:, j * 128 : (j + 1) * 128],
                    start=(j == 0), stop=(j == KC - 1),
                )
            # rowcombo = ln(counts) - ln(total) - r   (r already halved)
            rc4 = small.tile([KC, 128], f16)
            nc.vector.scalar_tensor_tensor(
                out=rc4[:], in0=ln4_h[:], scalar=-LN_TOTAL, in1=ps_r4[:],
                op0=AL.add, op1=AL.subtract,
            )
            # first g-chunk fills the PE gap while rc4 runs on the DVE
            ps_gs = []
            ps_g = psC.tile([128, C], f32, tag="g", bufs=3)
            nc.tensor.matmul(ps_g[:], lhsT=ztown[:, 0:128], rhs=pmt_h[:],
                             start=True, stop=True)
            ps_gs.append(ps_g)
            # broadcast rowcombo to [128, C] via PE outers
            ps_rc2 = psT1.tile([128, C], f32)
            for j in range(KC):
                nc.tensor.matmul(
                    ps_rc2[:, j * 128 : (j + 1) * 128],
                    lhsT=rowsel[:, j * 128 : (j + 1) * 128], rhs=rc4[:],
                    start=True, stop=True,
                )
            rc2_sb = sb.tile([128, C], f16)
            nc.scalar.copy(rc2_sb[:], ps_rc2[:])
            for k in range(1, KC):
                ps_g = psC.tile([128, C], f32, tag="g", bufs=3)
                nc.tensor.matmul(
                    ps_g[:], lhsT=ztown[:, k * 128 : (k + 1) * 128], rhs=pmt_h[:],
                    start=True, stop=True,
                )
                ps_gs.append(ps_g)

            # ---- phase C: final scores ----
            for k in range(KC):
                oc = sb.tile([128, C], f32, tag="oc", bufs=4)
                nc.vector.scalar_tensor_tensor(
                    out=oc[:], in0=ps_gs[k][:], scalar=qms[k][:], in1=rc2_sb[:],
                    op0=AL.add, op1=AL.add,
                )
                nc.sync.dma_start(out_d[k * 128 : (k + 1) * 128, :], oc[:])


_NC_CACHE = {}


def _get_nc():
    if "nc" not in _NC_CACHE:
        _NC_CACHE["nc"] = build_program()
    return _NC_CACHE["nc"]


def _consts():
    eye = np.eye(128, dtype=np.float32)
    iota = np.broadcast_to(np.arange(C, dtype=np.float16), (128, C))
    icol = (np.arange(128, dtype=np.float32)[:, None]
            + 128.0 * np.arange(KC, dtype=np.float32)[None, :]).astype(np.float32)
    colsel = np.zeros((128, KC * KC), dtype=np.float16)
    for j in range(KC):
        colsel[:, j * KC + j] = 1.0
    rowsel = np.zeros((KC, C), dtype=np.float16)
    for j in range(KC):
        rowsel[j, j * 128 : (j + 1) * 128] = 1.0
    return {
        "iota": np.ascontiguousarray(iota),
        "icol": icol,
        "identh": eye.astype(np.float16),
        "identf": eye,
        "epseye": (EPS * eye).astype(np.float32),
        "tceye": ((2.0 - EPS) * eye).astype(np.float32),
        "colsel": colsel,
        "rowsel": rowsel,
    }


def make_in_maps(z, y):
    z = np.asarray(z, dtype=np.float32)
    y = np.asarray(y).astype(np.float32)
    zh = np.ascontiguousarray(
        z.reshape(KA, 128, 128).transpose(1, 0, 2).astype(np.float16)
    )
    y16 = y.astype(np.float16)
    ypk = np.ascontiguousarray(y.reshape(KA, 128).T.astype(np.float32))
    ybc = np.ascontiguousarray(np.broadcast_to(y16[None, :], (128, B)))
    consts = _consts()
    shared = {f"zh{i}": np.ascontiguousarray(zh[:, i * KPT : (i + 1) * KPT, :])
              for i in range(NZH)}
    shared.update({"ypk": ypk, "ybc": ybc})
    shared.update(consts)
    in_maps = []
    for m in range(M):
        zs = z[m * BL : (m + 1) * BL]
        zs16 = zs.astype(np.float16)
        d = dict(shared)
        d["ztown"] = np.ascontiguousarray(zs16.T)
        d["zown"] = np.ascontiguousarray(
            zs16.reshape(KC, 128, 128).transpose(1, 0, 2)
        )
        in_maps.append(d)
    return in_maps


def kernel(z, y):
    z = np.asarray(z)
    y = np.asarray(y)
    assert z.shape == (B, D) and y.shape == (B,)
    nc = _get_nc()
    in_maps = make_in_maps(z, y)
    res = run_bass_kernel_spmd(nc, in_maps, list(range(M)), trace=False)
    out = np.concatenate([res.results[m]["scores"] for m in range(M)], axis=0)
    return out.astype(np.float32)


if __name__ == "__main__":
    rng = np.random.default_rng(0)
    z = rng.standard_normal((B, D), dtype=np.float32)
    y = rng.integers(0, C, size=B).astype(np.int32)
    out = kernel(z, y)
    print("scores:", out.shape, out.dtype, out[:2, :4])


# revision 13
# speedup vs baseline: 2.6010x; 1.2194x over previous
"""LDA head forward on 8 Trainium2 NeuronCores (Bass/Tile).

Fully replicated statistics — ZERO collectives. Rationale: the SPMD launch
skews core start times by tens of us, so any cross-core barrier (AllReduce)
makes every core wait for the last-launched one. Instead each core
redundantly computes the full-batch statistics (phase A over all B=4096
rows) and only the [B_l, C] scoring (phase C) is sharded. No core ever
waits on another.

Per core:
  phase A: for each of 32 row-chunks: one-hot via iota+is_equal (fp16),
           PE accumulates S1T = Z^T OH [D,C] and ZtZ = Z^T Z [D,D] in PSUM.
           counts come from the DVE free-dim reduction (accum_out) over
           class-partition one-hot planes [c_part, b] — no PE stream needed.
  phase B: pooled = (ZtZ - sum_c w_c S1_c S1_c^T)/total + eps*I with
           w_c = (counts_c + eps)/counts_c^2 (asymmetric Gram: no sqrt),
           precision P via Newton-Schulz in fp16 (X1 = 2c I - c^2 A direct),
           Pmt = P @ (S1T diag(1/counts)).
  phase C: scores[b,c] = lnprior_c - 0.5 r_c - 0.5 q_b + (Z P mean^T)[b,c]
           on the core's own 512 rows.

fp16 is used for all PE streams (1 cycle/row vs 4 for fp32 and for tf32 on
<256-col outputs). All accumulation stays fp32 in PSUM. Per-class scalars
that can overflow fp16 (1/counts = 1e5 for an empty class) are clamped; the
clamped values only ever multiply exact zeros (S1_c = 0 for empty c).
Row/col broadcasts are PE outer products with one-hot selector stationaries
(no gpsimd partition_broadcast on the critical path).
"""

import numpy as np

import concourse.bacc as bacc
import concourse.mybir as mybir
import concourse.tile as tile
from concourse.bass_utils import run_bass_kernel_spmd

f32 = mybir.dt.float32
f16 = mybir.dt.float16
AL = mybir.AluOpType
AF = mybir.ActivationFunctionType

M = 8            # cores
B = 4096
D = 128
C = 512
BL = B // M      # 512 rows per core
KC = BL // 128   # 4 own chunks of 128 rows
KA = B // 128    # 32 total chunks
EPS = 1e-5
TOTAL = float(B) + C * EPS
LN_TOTAL = float(np.log(np.float64(TOTAL)))
NS_C = 1.05      # Newton-Schulz init scale; X1 = 2c*I - c^2*A
NS_ITERS = 2
CLAMP = 60000.0  # fp16-safe cap for per-class reciprocals/weights
NZH = 8          # z arrives in NZH separate DMA tiles so PE can start early
KPT = KA // NZH  # chunks per z tile


def build_program():
    nc = bacc.Bacc("TRN2", target_bir_lowering=False, debug=False, num_devices=M)
    zh_d = [
        nc.dram_tensor(f"zh{i}", [128, KPT, 128], f16, kind="ExternalInput").ap()
        for i in range(NZH)
    ]
    ypk_d = nc.dram_tensor("ypk", [128, KA], f32, kind="ExternalInput").ap()
    zt_d = nc.dram_tensor("ztown", [D, BL], f16, kind="ExternalInput").ap()
    zown_d = nc.dram_tensor("zown", [128, KC, 128], f16, kind="ExternalInput").ap()
    iota_d = nc.dram_tensor("iota", [128, C], f16, kind="ExternalInput").ap()
    identh_d = nc.dram_tensor("identh", [128, 128], f16, kind="ExternalInput").ap()
    identf_d = nc.dram_tensor("identf", [128, 128], f32, kind="ExternalInput").ap()
    epseye_d = nc.dram_tensor("epseye", [128, 128], f32, kind="ExternalInput").ap()
    tceye_d = nc.dram_tensor("tceye", [128, 128], f32, kind="ExternalInput").ap()
    colsel_d = nc.dram_tensor("colsel", [128, KC * KC], f16, kind="ExternalInput").ap()
    rowsel_d = nc.dram_tensor("rowsel", [KC, C], f16, kind="ExternalInput").ap()
    out_d = nc.dram_tensor("scores", [BL, C], f32, kind="ExternalOutput").ap()

    with tile.TileContext(nc) as tc:
        _body(tc, out_d, zh_d, ypk_d, zt_d, zown_d, iota_d,
              identh_d, identf_d, epseye_d, tceye_d, colsel_d, rowsel_d)
    nc.compile()
    return nc


def _body(tc, out_d, zh_d, ypk_d, zt_d, zown_d, iota_d,
          identh_d, identf_d, epseye_d, tceye_d, colsel_d, rowsel_d):
    nc = tc.nc
    with (
        tc.tile_pool(name="const", bufs=1) as const,
        tc.tile_pool(name="io", bufs=1) as io,
        tc.tile_pool(name="sb", bufs=1) as sb,
        tc.tile_pool(name="small", bufs=1) as small,
    ):
        # ---- input DMAs: oh deps + bulk z first, tail-phase constants later ----
        iota = const.tile([128, C], f16)
        nc.sync.dma_start(iota[:], iota_d)
        ypk = const.tile([128, KA], f32)
        nc.sync.dma_start(ypk[:], ypk_d)
        zh = []
        for i in range(NZH):
            zt_i = io.tile([128, KPT, 128], f16, tag=f"zh{i}")
            nc.sync.dma_start(zt_i[:], zh_d[i])
            zh.append(zt_i)
        ident_h = const.tile([128, 128], f16)
        nc.sync.dma_start(ident_h[:], identh_d)
        ident_f = const.tile([128, 128], f32)
        nc.sync.dma_start(ident_f[:], identf_d)
        eps_eye = const.tile([128, 128], f32)
        nc.sync.dma_start(eps_eye[:], epseye_d)
        tc_eye = const.tile([128, 128], f32)
        nc.sync.dma_start(tc_eye[:], tceye_d)
        colsel = const.tile([128, KC * KC], f16)
        nc.sync.dma_start(colsel[:], colsel_d)
        rowsel = const.tile([KC, C], f16)
        nc.sync.dma_start(rowsel[:], rowsel_d)
        ztown = io.tile([D, BL], f16)
        nc.sync.dma_start(ztown[:], zt_d)
        zown = io.tile([128, KC, 128], f16)
        nc.sync.dma_start(zown[:], zown_d)

        # preload the Ln activation table while everything else runs
        tbl = small.tile([1, 1], f32)
        nc.scalar.activation(tbl[:], ident_f[0:1, 0:1], AF.Ln)
        ones_h = small.tile([128, 1], f16)
        nc.gpsimd.memset(ones_h[:], 1.0)

        with tc.tile_pool(name="psStats", bufs=1, space="PSUM") as psS:
            ps_s1t = psS.tile([128, C], f32)
            ps_ztz = psS.tile([128, 128], f32)

            # ---- phase A: stats over all B rows ----
            # counts: accumulate the one-hots on the DVE (fp16 tt-add runs in
            # 2x mode), then one ones^T matmul reduces partitions at the end.
            ohsum = sb.tile([128, C], f16)
            for k in range(KA):
                zc = zh[k // KPT][:, k % KPT, :]
                oh = sb.tile([128, C], f16, tag="oh", bufs=6)
                nc.vector.tensor_scalar(
                    out=oh[:], in0=iota[:], scalar1=ypk[:, k : k + 1], scalar2=None,
                    op0=AL.is_equal,
                )
                st, sp = k == 0, k == KA - 1
                nc.tensor.matmul(ps_ztz[:], lhsT=zc, rhs=zc, start=st, stop=sp)
                nc.tensor.matmul(ps_s1t[:], lhsT=zc, rhs=oh[:], start=st, stop=sp)
                if k == 0:
                    nc.vector.tensor_copy(ohsum[:], oh[:])
                else:
                    nc.vector.tensor_tensor(ohsum[:], ohsum[:], oh[:], op=AL.add)

            # counts row -> [128, KC] column layout via PE transposes
            with tc.tile_pool(name="psCnt", bufs=1, space="PSUM") as psQ:
                ps_cnt = psQ.tile([1, C], f32)
                nc.tensor.matmul(ps_cnt[:], lhsT=ones_h[:], rhs=ohsum[:], start=True, stop=True)
                cnt_row = small.tile([1, C], f32)
                nc.scalar.copy(cnt_row[:], ps_cnt[:])
                ps_c4 = psQ.tile([128, KC], f32)
                for j in range(KC):
                    nc.tensor.transpose(
                        ps_c4[:, j : j + 1], cnt_row[0:1, j * 128 : (j + 1) * 128],
                        ident_f[0:1, 0:1],
                    )
                cnt4 = small.tile([128, KC], f32)
                nc.vector.tensor_copy(cnt4[:], ps_c4[:])

            # ---- per-class scalar math [128, 4] ----
            cnts4 = small.tile([128, KC], f32)
            nc.vector.tensor_scalar(
                out=cnts4[:], in0=cnt4[:], scalar1=EPS, scalar2=None, op0=AL.add
            )
            rl_rc = small.tile([128, KC], f32)
            nc.vector.reciprocal(rl_rc[:], cnts4[:])
            rl_ln = small.tile([128, KC], f32)
            nc.scalar.activation(rl_ln[:], cnts4[:], AF.Ln)
            w4a = small.tile([128, KC], f32)
            nc.vector.tensor_scalar(
                out=w4a[:], in0=cnts4[:], scalar1=EPS, scalar2=None, op0=AL.add
            )
            w4b = small.tile([128, KC], f32)
            nc.vector.tensor_tensor(w4b[:], w4a[:], rl_rc[:], op=AL.mult)
            w4f = small.tile([128, KC], f32)
            nc.vector.tensor_tensor(w4f[:], w4b[:], rl_rc[:], op=AL.mult)

            s1_h = sb.tile([128, C], f16)
            nc.vector.tensor_copy(s1_h[:], ps_s1t[:])
            ztz_sb = sb.tile([128, 128], f32)
            nc.vector.tensor_copy(ztz_sb[:], ps_ztz[:])

        with tc.tile_pool(name="psB", bufs=1, space="PSUM") as psB:
            if True:
                # rcp and ln(counts) to [4, 128] class-chunk-partition layout
                ps_rc4 = psB.tile([KC, 128], f32)
                nc.tensor.transpose(ps_rc4[:], rl_rc[:], ident_f[:])
                ps_ln4 = psB.tile([KC, 128], f32)
                nc.tensor.transpose(ps_ln4[:], rl_ln[:], ident_f[:])
                rc4_h = small.tile([KC, 128], f16)
                nc.vector.tensor_scalar(
                    out=rc4_h[:], in0=ps_rc4[:], scalar1=CLAMP, scalar2=None, op0=AL.min
                )
                ln4_h = small.tile([KC, 128], f16)
                nc.scalar.copy(ln4_h[:], ps_ln4[:])

                # rc broadcast [128, C] via PE outer products (row-select stationary)
                ps_rcb = psB.tile([128, C], f32)
                for j in range(KC):
                    nc.tensor.matmul(
                        ps_rcb[:, j * 128 : (j + 1) * 128],
                        lhsT=rowsel[:, j * 128 : (j + 1) * 128], rhs=rc4_h[:],
                        start=True, stop=True,
                    )

                # W2 = sum_c w_c S1_c S1_c^T (asymmetric: scale one side by w)
                ps_w2 = psB.tile([128, 128], f32)
                for j in range(KC):
                    ps_tr = psB.tile([128, 128], f16, tag="tr", bufs=2)
                    nc.tensor.transpose(
                        ps_tr[:], s1_h[:, j * 128 : (j + 1) * 128], ident_h[:]
                    )
                    uj = sb.tile([128, 128], f16, tag="uj", bufs=2)
                    nc.scalar.copy(uj[:], ps_tr[:])
                    vj = sb.tile([128, 128], f16, tag="vj", bufs=2)
                    nc.vector.tensor_scalar(
                        out=vj[:], in0=ps_tr[:], scalar1=w4f[:, j : j + 1], scalar2=None,
                        op0=AL.mult,
                    )
                    nc.tensor.matmul(
                        ps_w2[:], lhsT=vj[:], rhs=uj[:], start=(j == 0), stop=(j == KC - 1)
                    )

                # mean^T in fp16 (meanT = S1T * (1/counts) column-broadcast)
                meanT = sb.tile([128, C], f16)
                nc.vector.tensor_tensor(meanT[:], s1_h[:], ps_rcb[:], op=AL.mult)

                # pooled covariance and Newton-Schulz X1 (both from pooled_f)
                pooled_f = sb.tile([128, 128], f32)
                nc.vector.tensor_tensor(pooled_f[:], ztz_sb[:], ps_w2[:], op=AL.subtract)
                pooled_h = sb.tile([128, 128], f16)
                nc.vector.scalar_tensor_tensor(
                    out=pooled_h[:], in0=pooled_f[:], scalar=1.0 / TOTAL,
                    in1=eps_eye[:], op0=AL.mult, op1=AL.add,
                )
                x_cur = sb.tile([128, 128], f16, tag="X", bufs=2)
                nc.vector.scalar_tensor_tensor(
                    out=x_cur[:], in0=pooled_f[:], scalar=-1.0 / TOTAL,
                    in1=tc_eye[:], op0=AL.mult, op1=AL.add,
                )

        with tc.tile_pool(name="psNS", bufs=1, space="PSUM") as psN:
            for i in range(NS_ITERS):
                ps_t = psN.tile([128, 128], f32, tag="T", bufs=1)
                nc.tensor.matmul(ps_t[:], lhsT=pooled_h[:], rhs=x_cur[:], start=True, stop=True)
                t_h = sb.tile([128, 128], f16, tag="Th", bufs=2)
                nc.scalar.copy(t_h[:], ps_t[:])
                ps_u = psN.tile([128, 128], f32, tag="U", bufs=1)
                nc.tensor.matmul(ps_u[:], lhsT=x_cur[:], rhs=t_h[:], start=True, stop=True)
                x_new = sb.tile([128, 128], f16, tag="X", bufs=2)
                nc.vector.scalar_tensor_tensor(
                    out=x_new[:], in0=x_cur[:], scalar=2.0, in1=ps_u[:],
                    op0=AL.mult, op1=AL.subtract,
                )
                x_cur = x_new

        # ---- phase C setup: q_b early (only needs P), then Pmt ----
        with (
            tc.tile_pool(name="psT1", bufs=1, space="PSUM") as psT1,
            tc.tile_pool(name="psC", bufs=1, space="PSUM") as psC,
        ):
            qms = []
            for k in range(KC):
                ps_zp = psC.tile([128, 128], f32, tag="zp", bufs=2)
                nc.tensor.matmul(
                    ps_zp[:], lhsT=ztown[:, k * 128 : (k + 1) * 128], rhs=x_cur[:],
                    start=True, stop=True,
                )
                zpz = sb.tile([128, 128], f16, tag="zpz", bufs=2)
                qm = small.tile([128, 1], f32, tag="qm", bufs=KC)
                nc.vector.scalar_tensor_tensor(
                    out=zpz[:], in0=ps_zp[:], scalar=-0.5, in1=zown[:, k, :],
                    op0=AL.mult, op1=AL.mult, accum_out=qm[:],
                )
                qms.append(qm)

            ps_pmt = psT1.tile([128, C], f32)
            nc.tensor.matmul(ps_pmt[:], lhsT=x_cur[:], rhs=meanT[:], start=True, stop=True)
            pmt_h = sb.tile([128, C], f16)
            nc.scalar.copy(pmt_h[:], ps_pmt[:])
            # r_c = colsum(0.5 * meanT . Pmt) into [4, 128] via col-select
            prod_h = sb.tile([128, C], f16)
            nc.vector.scalar_tensor_tensor(
                out=prod_h[:], in0=ps_pmt[:], scalar=0.5, in1=meanT[:],
                op0=AL.mult, op1=AL.mult,
            )
            ps_r4 = psT1.tile([KC, 128], f32)
            for j in range(KC):
                nc.tensor.matmul(
                    ps_r4[:], lhsT=colsel[:, j * KC : (j + 1) * KC],
                    rhs=prod_h[:, j * 128 : (j + 1) * 128],
                    start=(j == 0), stop=(j == KC - 1),
                )
            # rowcombo = ln(counts) - ln(total) - r   (r already halved)
            rc4 = small.tile([KC, 128], f16)
            nc.vector.scalar_tensor_tensor(
                out=rc4[:], in0=ln4_h[:], scalar=-LN_TOTAL, in1=ps_r4[:],
                op0=AL.add, op1=AL.subtract,
            )
            # first g-chunk fills the PE gap while rc4 runs on the DVE
            ps_gs = []
            ps_g = psC.tile([128, C], f32, tag="g", bufs=3)
            nc.tensor.matmul(ps_g[:], lhsT=ztown[:, 0:128], rhs=pmt_h[:],
                             start=True, stop=True)
            ps_gs.append(ps_g)
            # broadcast rowcombo to [128, C] via PE outers
            ps_rc2 = psT1.tile([128, C], f32)
            for j in range(KC):
                nc.tensor.matmul(
                    ps_rc2[:, j * 128 : (j + 1) * 128],
                    lhsT=rowsel[:, j * 128 : (j + 1) * 128], rhs=rc4[:],
                    start=True, stop=True,
                )
            rc2_sb = sb.tile([128, C], f16)
            nc.scalar.copy(rc2_sb[:], ps_rc2[:])
            for k in range(1, KC):
                ps_g = psC.tile([128, C], f32, tag="g", bufs=3)
                nc.tensor.matmul(
                    ps_g[:], lhsT=ztown[:, k * 128 : (k + 1) * 128], rhs=pmt_h[:],
                    start=True, stop=True,
                )
                ps_gs.append(ps_g)

            # ---- phase C: final scores ----
            for k in range(KC):
                oc = sb.tile([128, C], f32, tag="oc", bufs=4)
                nc.vector.scalar_tensor_tensor(
                    out=oc[:], in0=ps_gs[k][:], scalar=qms[k][:], in1=rc2_sb[:],
                    op0=AL.add, op1=AL.add,
                )
                nc.sync.dma_start(out_d[k * 128 : (k + 1) * 128, :], oc[:])


_NC_CACHE = {}


def _get_nc():
    if "nc" not in _NC_CACHE:
        _NC_CACHE["nc"] = build_program()
    return _NC_CACHE["nc"]


def _consts():
    eye = np.eye(128, dtype=np.float32)
    iota = np.broadcast_to(np.arange(C, dtype=np.float16), (128, C))
    colsel = np.zeros((128, KC * KC), dtype=np.float16)
    for j in range(KC):
        colsel[:, j * KC + j] = 1.0
    rowsel = np.zeros((KC, C), dtype=np.float16)
    for j in range(KC):
        rowsel[j, j * 128 : (j + 1) * 128] = 1.0
    return {
        "iota": np.ascontiguousarray(iota),
        "identh": eye.astype(np.float16),
        "identf": eye,
        "epseye": (EPS * eye).astype(np.float32),
        "tceye": ((2.0 - EPS) * eye).astype(np.float32),
        "colsel": colsel,
        "rowsel": rowsel,
    }


def make_in_maps(z, y):
    z = np.asarray(z, dtype=np.float32)
    y = np.asarray(y).astype(np.float32)
    zh = np.ascontiguousarray(
        z.reshape(KA, 128, 128).transpose(1, 0, 2).astype(np.float16)
    )
    ypk = np.ascontiguousarray(y.reshape(KA, 128).T.astype(np.float32))
    consts = _consts()
    shared = {f"zh{i}": np.ascontiguousarray(zh[:, i * KPT : (i + 1) * KPT, :])
              for i in range(NZH)}
    shared.update({"ypk": ypk})
    shared.update(consts)
    in_maps = []
    for m in range(M):
        zs = z[m * BL : (m + 1) * BL]
        zs16 = zs.astype(np.float16)
        d = dict(shared)
        d["ztown"] = np.ascontiguousarray(zs16.T)
        d["zown"] = np.ascontiguousarray(
            zs16.reshape(KC, 128, 128).transpose(1, 0, 2)
        )
        in_maps.append(d)
    return in_maps


def kernel(z, y):
    z = np.asarray(z)
    y = np.asarray(y)
    assert z.shape == (B, D) and y.shape == (B,)
    nc = _get_nc()
    in_maps = make_in_maps(z, y)
    res = run_bass_kernel_spmd(nc, in_maps, list(range(M)), trace=False)
    out = np.concatenate([res.results[m]["scores"] for m in range(M)], axis=0)
    return out.astype(np.float32)


if __name__ == "__main__":
    rng = np.random.default_rng(0)
    z = rng.standard_normal((B, D), dtype=np.float32)
    y = rng.integers(0, C, size=B).astype(np.int32)
    out = kernel(z, y)
    print("scores:", out.shape, out.dtype, out[:2, :4])
